# revision 6
# baseline (speedup 1.0000x reference)
"""Trainium2 Bass kernel for fake-quant (W8A8) linear: y = fq_tok(x) @ fq_ch(w).T + b.

Full shapes: x [4, 2048, 4096] f32, w [4096, 4096] f32, b [4096] f32.
Sharding over 8 cores: 2 token groups x 4 out-channel groups.
Per core: x_sh [4096, 4096], w_sh [1024, 4096], b_sh [1024] -> y_sh [4096, 1024].

Key idea: quantized values are integers in [-127, 127], exactly representable
in bf16, so the matmul runs on the PE array in bf16 (full rate) with fp32 PSUM
accumulation - numerically equivalent to the fp32 reference einsum on the
dequantized values.  Scales are applied in the fp32 epilogue.

Rounding: round-half-to-even via the fp32 magic-constant trick
(v + 1.5*2^23 rounds mantissa to integer; subtract again afterwards),
matching jnp.round.  Clipping to [-128, 127] is a no-op by construction
(|x|/s <= 127 when s = amax/127) so it is skipped.

Schedule (v3).  Steady state is PE-bound (measured: 219ns per 512-wide
matmul, 58ns per 128x128 transpose -> ~16us/tile vs ~11.7us/tile on each of
ACT/DVE), so the whole game is keeping the in-order PE queue fed during the
head while ~118us of w+x quantize work drains through ACT+DVE:
 - qwT is split into two channel-half tiles; tile matmuls for channels
   [0,512) only wait on w-tiles 0-3.
 - cb1 (channels [512,1024)) matmul blocks are emitted DEFER=3 tiles behind
   cb0 blocks, so the PE never blocks in-order on qwT half 1 while w4-7 are
   still quantizing.
 - x0 is emitted before the weight phase; x quantize passes are split into
   K-halves for lower latency; w magic-subs alternate ACT/DVE so neither
   engine is the sole head bottleneck.
 - sw_b broadcast is split per channel half (cb0 epilogues only wait on
   w0-3's scales); bb_b is loaded up front.
 - the 4 per-tile PSUM->SBUF transpose-drain copies are split 2/2 between
   ACT and DVE.
"""

from contextlib import ExitStack

import numpy as np

import concourse.bass as bass
import concourse.mybir as mybir
import concourse.tile as tile
from concourse import bacc
from concourse.masks import make_identity

P = 128
MAGIC = 12582912.0  # 1.5 * 2**23
QMAX = 127.0
EPS = 1e-8

# full problem shapes (hardcoded per harness contract)
B, S, D_IN, D_OUT = 4, 2048, 4096, 4096
TOK = B * S  # 8192
TOK_GROUPS = 2
CH_GROUPS = 4
T_SH = TOK // TOK_GROUPS  # 4096 tokens per core
O_SH = D_OUT // CH_GROUPS  # 1024 channels per core

DEFER = 3  # cb1 matmul blocks trail cb0 by this many tiles


def build_nc(T, K, O, nch=512):
    """Build the per-core Bass program: x[T,K], w[O,K], b[O] -> y[T,O]."""
    f32 = mybir.dt.float32
    bf16 = mybir.dt.bfloat16
    Copy = mybir.ActivationFunctionType.Copy
    Alu = mybir.AluOpType
    AxX = mybir.AxisListType.X

    assert T % P == 0 and K % P == 0 and O % P == 0
    TT, KB, WT = T // P, K // P, O // P
    NCH = min(nch, O)
    CB = O // NCH  # channel halves (2)
    WPH = WT // CB  # w tiles per channel half (4)
    KH = K // 2  # K-half for latency-split quantize passes

    nc = bacc.Bacc("TRN2", target_bir_lowering=False, debug=False)
    x_ap = nc.dram_tensor("x", [T, K], f32, kind="ExternalInput").ap()
    w_ap = nc.dram_tensor("w", [O, K], f32, kind="ExternalInput").ap()
    b_ap = nc.dram_tensor("b", [O], f32, kind="ExternalInput").ap()
    y_ap = nc.dram_tensor("y", [T, O], f32, kind="ExternalOutput").ap()

    with tile.TileContext(nc) as tc, ExitStack() as ctx:
        singles = ctx.enter_context(tc.tile_pool(name="singles", bufs=1))
        bigf32 = ctx.enter_context(tc.tile_pool(name="bigf32", bufs=3))
        rnd = ctx.enter_context(tc.tile_pool(name="rnd", bufs=3))
        qpool = ctx.enter_context(tc.tile_pool(name="qpool", bufs=2))
        qtpool = ctx.enter_context(tc.tile_pool(name="qtpool", bufs=5))
        stats = ctx.enter_context(tc.tile_pool(name="stats", bufs=24))
        opool = ctx.enter_context(tc.tile_pool(name="opool", bufs=3))
        psum_pool = ctx.enter_context(tc.tile_pool(name="psum", bufs=4, space="PSUM"))
        tpsum = ctx.enter_context(tc.tile_pool(name="tpsum", bufs=3, space="PSUM"))
        dram = ctx.enter_context(tc.tile_pool(name="dram", bufs=1, space="DRAM"))

        # resident: transposed quantized weights (split in two channel
        # halves so early matmuls only depend on half the weight phase)
        # qwT_h[cb][f, k, c] = qw[cb*NCH + c, k*128+f]
        qwT_h = [
            singles.tile([P, KB, NCH], bf16, name=f"qwT_h{i}") for i in range(CB)
        ]
        sw_b = singles.tile([P, O], f32)
        bb_b = singles.tile([P, O], f32)
        sw_dram = dram.tile([O, 1], f32)
        ident = singles.tile([P, P], bf16)
        make_identity(nc, ident)

        # bias broadcast has no dependencies - up front
        nc.sync.dma_start(
            out=bb_b,
            in_=bass.AP(tensor=b_ap.tensor, offset=b_ap.offset, ap=[[0, P], [1, O]]),
        )

        TG = min(8, KB)  # k-blocks per PE-transpose psum group (8*128 bf16 = one bank)

        def pe_transpose(q_sbuf, dst, tag, dst_col_base=0):
            # q_sbuf [P, K] bf16 -> dst [P, KB, *] slice view with
            # dst[f, k, dst_col_base + c] = q_sbuf[c, k*128+f]
            # PE transposes into PSUM; drain copies split between ACT (even
            # groups) and DVE (odd groups) to balance the two engines.
            for g in range(KB // TG):
                tp = tpsum.tile([P, TG, P], bf16, tag="tp", name=f"tp_{tag}_{g}")
                for j in range(TG):
                    kb = g * TG + j
                    nc.tensor.transpose(
                        tp[:, j, :], q_sbuf[:, kb * P : (kb + 1) * P], ident
                    )
                dst_sl = dst[:, g * TG : (g + 1) * TG,
                             dst_col_base : dst_col_base + P]
                if g % 2 == 0:
                    nc.scalar.activation(out=dst_sl, in_=tp, func=Copy)
                else:
                    nc.vector.tensor_copy(dst_sl, tp)

        def quantize(src_t, q_t, s_t, tag, dve_magic=False):
            # per-row amax -> scale (s_t) -> 1/s, then round src*(1/s) to
            # integers in q_t (bf16), via the magic trick in K-halves.
            amax = stats.tile([P, 1], f32, tag="st", name=f"amax_{tag}")
            nc.vector.reduce_max(
                out=amax, in_=src_t, axis=AxX, apply_absolute_value=True
            )
            s_col = s_t[:, 0:1]
            nc.vector.tensor_scalar(
                out=s_col, in0=amax, scalar1=1.0 / QMAX, scalar2=EPS,
                op0=Alu.mult, op1=Alu.max,
            )
            r_t = stats.tile([P, 1], f32, tag="st", name=f"recip_{tag}")
            nc.vector.reciprocal(out=r_t, in_=s_col)
            for h in range(2):
                sl = slice(h * KH, (h + 1) * KH)
                t_t = rnd.tile([P, KH], f32, tag="rnd", name=f"t_{tag}_{h}")
                nc.scalar.activation(
                    out=t_t, in_=src_t[:, sl], func=Copy, bias=MAGIC,
                    scale=r_t[:, 0:1],
                )
                if dve_magic:
                    nc.vector.tensor_scalar(
                        out=q_t[:, sl], in0=t_t, scalar1=-MAGIC, scalar2=None,
                        op0=Alu.add,
                    )
                else:
                    nc.scalar.activation(
                        out=q_t[:, sl], in_=t_t, func=Copy, bias=-MAGIC, scale=1.0
                    )

        def process_w_tile(wt):
            w_t = bigf32.tile([P, K], f32, tag="big", name=f"w_{wt}")
            nc.sync.dma_start(out=w_t, in_=w_ap[wt * P : (wt + 1) * P, :])
            sw = stats.tile([P, 1], f32, tag="st", name=f"sw_{wt}")
            qw = qpool.tile([P, K], bf16, tag="q", name=f"qw_{wt}")
            # alternate the magic-sub engine so ACT and DVE share the head
            quantize(w_t, qw, sw, f"w{wt}", dve_magic=(wt % 2 == 1))
            cb, sub = divmod(wt, WPH)
            pe_transpose(qw, qwT_h[cb], f"w{wt}", dst_col_base=sub * P)
            nc.sync.dma_start(out=sw_dram[wt * P : (wt + 1) * P, :], in_=sw)

        def bcast_sw_half(cb):
            nc.sync.dma_start(
                out=sw_b[:, cb * NCH : (cb + 1) * NCH],
                in_=bass.AP(
                    tensor=sw_dram.tensor,
                    offset=sw_dram.offset + cb * NCH,
                    ap=[[0, P], [1, NCH]],
                ),
            )

        def load_quant_transpose_x(tt):
            x_t = bigf32.tile([P, K], f32, tag="big", name=f"x_{tt}")
            nc.sync.dma_start(out=x_t, in_=x_ap[tt * P : (tt + 1) * P, :])
            sx = stats.tile([P, 1], f32, tag="st", name=f"sx_{tt}")
            qx = qpool.tile([P, K], bf16, tag="q", name=f"qx_{tt}")
            quantize(x_t, qx, sx, f"x{tt}")
            qxT = qtpool.tile([P, KB, P], bf16)  # qxT[f, k, t] = qx[t, k*128+f]
            pe_transpose(qx, qxT, f"x{tt}")
            return sx, qxT

        def matmul_half(tt, cb, sx, qxT):
            psum = psum_pool.tile([P, NCH], f32, tag="psum", name=f"ps_{tt}_{cb}")
            for k in range(KB):
                nc.tensor.matmul(
                    psum,
                    qxT[:, k, :],
                    qwT_h[cb][:, k, :],
                    start=(k == 0),
                    stop=(k == KB - 1),
                )
            o1 = opool.tile([P, NCH], f32, tag="o", name=f"o1_{tt}_{cb}")
            nc.vector.scalar_tensor_tensor(
                out=o1, in0=psum, scalar=sx[:, 0:1],
                in1=sw_b[:, cb * NCH : (cb + 1) * NCH],
                op0=Alu.mult, op1=Alu.mult,
            )
            o2 = opool.tile([P, NCH], f32, tag="o", name=f"o2_{tt}_{cb}")
            nc.vector.tensor_add(
                out=o2, in0=o1, in1=bb_b[:, cb * NCH : (cb + 1) * NCH]
            )
            nc.sync.dma_start(
                out=y_ap[tt * P : (tt + 1) * P, cb * NCH : (cb + 1) * NCH],
                in_=o2,
            )

        # ---- head: x0 first (fills DVE/ACT/PE before w arrives), then the
        # first weight half, then interleave remaining w with early x tiles.
        xrec = {}
        xrec[0] = load_quant_transpose_x(0)
        for wt in range(WPH):
            process_w_tile(wt)
        bcast_sw_half(0)
        xrec[1] = load_quant_transpose_x(1)
        matmul_half(0, 0, *xrec[0])
        process_w_tile(WPH)
        process_w_tile(WPH + 1)
        xrec[2] = load_quant_transpose_x(2)
        matmul_half(1, 0, *xrec[1])
        process_w_tile(WPH + 2)
        process_w_tile(WPH + 3)
        bcast_sw_half(1)

        # ---- steady: emit X(t+1), MM(t,0), MM(t-DEFER,1); cb1 trails so the
        # in-order PE queue never blocks on late inputs.
        for t in range(2, TT):
            if t + 1 < TT:
                xrec[t + 1] = load_quant_transpose_x(t + 1)
            matmul_half(t, 0, *xrec[t])
            tb = t - DEFER
            if tb >= 0:
                matmul_half(tb, 1, *xrec[tb])
                del xrec[tb]
        for tb in range(TT - DEFER, TT):
            matmul_half(tb, 1, *xrec[tb])
    nc.compile()
    return nc


_cached_nc = None


def _get_nc():
    global _cached_nc
    if _cached_nc is None:
        _cached_nc = build_nc(T_SH, D_IN, O_SH)
    return _cached_nc


def kernel(x: np.ndarray, w: np.ndarray, b: np.ndarray, _trace=False):
    from concourse.bass_utils import run_bass_kernel_spmd

    assert x.shape == (B, S, D_IN) and w.shape == (D_OUT, D_IN) and b.shape == (D_OUT,)
    x2 = np.ascontiguousarray(x.reshape(TOK, D_IN), dtype=np.float32)
    w2 = np.ascontiguousarray(w, dtype=np.float32)
    b2 = np.ascontiguousarray(b, dtype=np.float32)

    in_maps = []
    for core in range(8):
        tg, cg = divmod(core, CH_GROUPS)
        in_maps.append(
            {
                "x": np.ascontiguousarray(x2[tg * T_SH : (tg + 1) * T_SH]),
                "w": np.ascontiguousarray(w2[cg * O_SH : (cg + 1) * O_SH]),
                "b": np.ascontiguousarray(b2[cg * O_SH : (cg + 1) * O_SH]),
            }
        )

    nc = _get_nc()
    res = run_bass_kernel_spmd(nc, in_maps, core_ids=list(range(8)), trace=_trace)

    y = np.empty((TOK, D_OUT), dtype=np.float32)
    for core in range(8):
        tg, cg = divmod(core, CH_GROUPS)
        y[tg * T_SH : (tg + 1) * T_SH, cg * O_SH : (cg + 1) * O_SH] = res.results[
            core
        ]["y"]
    if _trace:
        kernel._last_results = res
    return y.reshape(B, S, D_OUT)
